# revision 5
# baseline (speedup 1.0000x reference)
"""Distributed causal multi-head attention (RoPE) for 8 TRN2 NeuronCores.

Problem: B=4, S=2048, D=2048, H=16 heads, DH=128.
Sharding: 2D - data-parallel over the 4 batches x tensor-parallel over 2
head-groups of 8 heads (Megatron-style: Wqkv column-sharded per head
group, Wo row-sharded).  Core c handles batch c//2, head group c%2.
Each core returns a partial output projection [S, D]; the host sums the
two group partials per batch (the "all-reduce") and stacks batches.

v2: all matmul inputs bf16 (fp32 PSUM accumulation), fully SBUF-resident
(no DRAM spill of Q/K/V), and a V-stationary PV stage:
  stage 1: QKV projection from xT (d-major, bf16).  RoPE fused into the
           PSUM eviction for Q/K (DVE cross-partition reads, sign folded
           into the sin table); Q/K/V stay resident in SBUF as bf16.
           x is DMA'd in token-chunk order so the first matmul chain
           starts ~8us in instead of starving ~50us on the full load.
  stage 2: per head: scoresT[k,q] = K-tile.T x Q (bf16, causally trimmed
           at 128 granularity), exp via ACT (scale fused) into bf16
           tiles, triangular mask on diagonal blocks only.  PV is
           V-stationary: zT[dh,q] += V_kt.T @ st (one LDWEIGHTS per
           K-tile, N=512 moving) - no z transposes, LDW fully hidden.
           Softmax denominator: DVE pairwise tree-sum of the exp'd
           tiles, a 1-column ones matmul for the partition reduction,
           DVE reciprocal, gpsimd partition_broadcast, DVE multiply.
           QK of chunk qr+1 is emitted before PV of chunk qr (PE is
           in-order; this hides the ACT exp latency).
  stage 3: output projection out = sum_h zT_h.T x WoT_h (bf16, N=512).

Baseline (v1, fp32r + DRAM spill + st-stationary PV): ~727-736 us.
"""

import sys

if '/opt/trn_rl_repo' not in sys.path:
    sys.path.insert(0, '/opt/trn_rl_repo')

import math

import ml_dtypes
import numpy as np

B, S, D, H, DH = 4, 2048, 2048, 16, 128
BASE = 10000.0
P = 128
NT = S // P          # 16 token tiles of 128
NC512 = S // 512     # 4 token chunks of 512
NDM = D // P         # 16 d_model chunks
HG = 8               # heads per group
SCALE = 1.0 / math.sqrt(DH)

_CACHE = {}


def _build_program():
    import concourse.bacc as bacc
    import concourse.mybir as mybir
    from concourse.tile import TileContext

    F32 = mybir.dt.float32
    F32R = mybir.dt.float32r
    BF16 = mybir.dt.bfloat16
    EXP = mybir.ActivationFunctionType.Exp

    nc = bacc.Bacc('TRN2', target_bir_lowering=False, debug=False, num_devices=8)

    # ---- DRAM I/O (all weights/activations bf16; tables f32) ----
    xT = nc.dram_tensor('xT', [P, NC512, NDM, 512], BF16, kind='ExternalInput').ap()
    wqkT = nc.dram_tensor('wqkT', [2 * HG, P, NDM, P], BF16, kind='ExternalInput').ap()
    wvT = nc.dram_tensor('wvT', [P, NDM, HG * P], BF16, kind='ExternalInput').ap()
    woT = nc.dram_tensor('woT', [P, HG, D], BF16, kind='ExternalInput').ap()
    cosT = nc.dram_tensor('cosT', [P, S], F32, kind='ExternalInput').ap()
    sinP = nc.dram_tensor('sinP', [P, S], F32, kind='ExternalInput').ap()
    maskT = nc.dram_tensor('maskT', [P, P], BF16, kind='ExternalInput').ap()
    onescol = nc.dram_tensor('onescol', [P, 1], F32R, kind='ExternalInput').ap()
    out = nc.dram_tensor('out', [NT, P, D], F32, kind='ExternalOutput').ap()

    with TileContext(nc) as tc:
        # Persistent across stages 1-2: Q/K feature blocks and V token tiles.
        with tc.tile_pool(name='qk', bufs=1) as qkpool, \
             tc.tile_pool(name='v', bufs=1) as vpool:
            qksb = [qkpool.tile([P, S], BF16, name=f'qk{fb}') for fb in range(2 * HG)]
            vsb = [vpool.tile([P, HG * P], BF16, name=f'v{tt}') for tt in range(NT)]

            # ================= stage 1: QKV projection =================
            with tc.tile_pool(name='s1x', bufs=1) as xpool:
                xsb = xpool.tile([P, NC512, NDM, 512], BF16)
                with tc.tile_pool(name='s1t', bufs=1) as tpool, \
                     tc.tile_pool(name='s1w', bufs=2) as wpool, \
                     tc.tile_pool(name='s1e', bufs=2) as epool, \
                     tc.tile_pool(name='s1p', bufs=4, space='PSUM') as ppool:
                    cos_sb = tpool.tile([P, S], F32)
                    sin_sb = tpool.tile([P, S], F32)
                    for fb in range(2 * HG):
                        wsb = wpool.tile([P, NDM, P], BF16, tag='w', name=f'wqk{fb}')
                        nc.sync.dma_start(wsb[:], wqkT[fb])
                        if fb == 0:
                            # x in token-chunk order so (fb0, tcn0) starts asap
                            nc.sync.dma_start(xsb[:, 0], xT[:, 0])
                            nc.sync.dma_start(cos_sb[:], cosT[:])
                            nc.sync.dma_start(sin_sb[:], sinP[:])
                            for tcn in range(1, NC512):
                                nc.sync.dma_start(xsb[:, tcn], xT[:, tcn])
                        for tcn in range(NC512):
                            ts = slice(tcn * 512, tcn * 512 + 512)
                            ps = ppool.tile([P, 512], F32, tag='pqk', name=f'pqk_{fb}_{tcn}')
                            for o in range(NDM):
                                nc.tensor.matmul(ps[:], wsb[:, o, :], xsb[:, tcn, o, :],
                                                 start=(o == 0), stop=(o == NDM - 1))
                            # RoPE fused eviction (sign folded in sinP)
                            t1 = epool.tile([P, 512], F32, tag='tt', name=f't1_{fb}_{tcn}')
                            t2 = epool.tile([P, 512], F32, tag='tt', name=f't2_{fb}_{tcn}')
                            nc.vector.tensor_mul(t1[:], ps[:], cos_sb[:, ts])
                            nc.vector.tensor_mul(t2[0:64, :], ps[64:128, :], sin_sb[0:64, ts])
                            nc.vector.tensor_mul(t2[64:128, :], ps[0:64, :], sin_sb[64:128, ts])
                            nc.vector.tensor_add(qksb[fb][:, ts], t1[:], t2[:])

                    # --- V blocks, token-major, N=256 ---
                    for vc in range(4):
                        vs = slice(vc * 256, vc * 256 + 256)
                        wv = wpool.tile([P, NDM, 256], BF16, tag='wv', name=f'wv{vc}')
                        for o in range(NDM):
                            nc.sync.dma_start(wv[:, o, :], wvT[:, o, vs])
                        for tt in range(NT):
                            psv = ppool.tile([P, 256], F32, tag='pv', name=f'pv_{vc}_{tt}')
                            for o in range(NDM):
                                nc.tensor.matmul(
                                    psv[:],
                                    xsb[:, tt // 4, o, (tt % 4) * P:(tt % 4 + 1) * P],
                                    wv[:, o, :],
                                    start=(o == 0), stop=(o == NDM - 1))
                            nc.scalar.copy(vsb[tt][:, vs], psv[:])

            # ================= stage 2 + 3 =================
            with tc.tile_pool(name='zt', bufs=1) as ztpool:
                zT = [ztpool.tile([P, S], BF16, name=f'zT{h}') for h in range(HG)]

                with tc.tile_pool(name='s2c', bufs=1) as cpool, \
                     tc.tile_pool(name='s2st', bufs=2) as stpool, \
                     tc.tile_pool(name='s2dn', bufs=8) as dnpool, \
                     tc.tile_pool(name='s2dp', bufs=2) as dppool, \
                     tc.tile_pool(name='s2rc', bufs=1) as rcpool, \
                     tc.tile_pool(name='s2ps', bufs=4, space='PSUM') as sppool, \
                     tc.tile_pool(name='s2pz', bufs=2, space='PSUM') as zppool, \
                     tc.tile_pool(name='s2pd', bufs=2, space='PSUM') as dpppool:
                    msk = cpool.tile([P, P], BF16)
                    nc.sync.dma_start(msk[:], maskT[:])
                    ones_sb = cpool.tile([P, 1], F32R)
                    nc.sync.dma_start(ones_sb[:], onescol[:])

                    for h in range(HG):
                        qt_h = qksb[h]
                        kt_h = qksb[HG + h]
                        st = [[None] * NT for _ in range(NC512)]

                        def emit_qk(qr, h=h, qt_h=qt_h, kt_h=kt_h, st=st):
                            base = qr * 512
                            for kt in range(4 * qr + 4):
                                d = kt - 4 * qr
                                qoff = 0 if d < 0 else 128 * d
                                sps = sppool.tile([P, 512], F32, tag='sps',
                                                  name=f'sps_{h}_{qr}_{kt}')
                                nc.tensor.matmul(sps[:, qoff:512],
                                                 kt_h[:, kt * P:(kt + 1) * P],
                                                 qt_h[:, base + qoff:base + 512],
                                                 start=True, stop=True)
                                stt = stpool.tile([P, 512], BF16, tag=f'st{kt}',
                                                  name=f'st_{h}_{qr}_{kt}')
                                nc.scalar.activation(stt[:, qoff:512], sps[:, qoff:512],
                                                     EXP, scale=SCALE)
                                if d >= 0:
                                    # triangular mask on the diagonal 128-block only
                                    nc.vector.tensor_mul(stt[:, qoff:qoff + P],
                                                         stt[:, qoff:qoff + P], msk[:])
                                st[qr][kt] = stt

                        emit_qk(0)
                        for qr in range(NC512):
                            if qr + 1 < NC512:
                                emit_qk(qr + 1)   # QK(qr+1) on PE while ACT exps it
                            nkt = 4 * qr + 4
                            # --- PV, V-stationary: zT accum [dh, q] ---
                            zps = zppool.tile([P, 512], F32, tag='zps',
                                              name=f'zps_{h}_{qr}')
                            for kt in range(nkt):
                                d = kt - 4 * qr
                                qoff = 0 if d < 0 else 128 * d
                                nc.tensor.matmul(zps[:, qoff:512],
                                                 vsb[kt][:, h * P:(h + 1) * P],
                                                 st[qr][kt][:, qoff:512],
                                                 start=(kt == 0), stop=(kt == nkt - 1))
                            # --- denominator: bf16 pairwise tree over full tiles,
                            #     then the 3 trimmed diagonal tiles in f32 ---
                            fulls = [st[qr][kt][:, 0:512] for kt in range(4 * qr + 1)]
                            lvl = 0
                            while len(fulls) > 2:
                                nxt = []
                                for i in range(0, len(fulls) - 1, 2):
                                    tsum = dnpool.tile([P, 512], BF16, tag='dnb',
                                                       name=f'dn_{h}_{qr}_{lvl}_{i}')
                                    nc.vector.tensor_add(tsum[:], fulls[i], fulls[i + 1])
                                    nxt.append(tsum[:])
                                    lvl += 1
                                if len(fulls) % 2:
                                    nxt.append(fulls[-1])
                                fulls = nxt
                            dp = dppool.tile([P, 512], F32R, tag='dp',
                                             name=f'dp_{h}_{qr}')
                            if len(fulls) == 2:
                                nc.vector.tensor_add(dp[:], fulls[0], fulls[1])
                            else:
                                nc.vector.tensor_copy(dp[:], fulls[0])
                            for dd in range(1, 4):
                                qoff = 128 * dd
                                nc.vector.tensor_add(dp[:, qoff:512], dp[:, qoff:512],
                                                     st[qr][4 * qr + dd][:, qoff:512])
                            den_ps = dpppool.tile([1, 512], F32, tag='denp',
                                                  name=f'den_{h}_{qr}')
                            nc.tensor.matmul(den_ps[:], ones_sb[:], dp[:],
                                             start=True, stop=True)
                            rcp = rcpool.tile([1, 512], F32, tag='rcp',
                                              name=f'rcp_{h}_{qr}')
                            nc.vector.reciprocal(rcp[:], den_ps[:])
                            rcpb = rcpool.tile([P, 512], F32, tag='rcpb',
                                               name=f'rcpb_{h}_{qr}')
                            nc.gpsimd.partition_broadcast(rcpb[:], rcp[:])
                            nc.vector.tensor_mul(zT[h][:, qr * 512:(qr + 1) * 512],
                                                 zps[:], rcpb[:])

                # ================= stage 3: output projection =================
                with tc.tile_pool(name='s3o', bufs=4) as ospool, \
                     tc.tile_pool(name='s3w', bufs=2) as wopool, \
                     tc.tile_pool(name='s3p', bufs=4, space='PSUM') as oppool:
                    for ec in range(NC512):
                        es = slice(ec * 512, ec * 512 + 512)
                        wo = wopool.tile([P, HG, 512], BF16, tag='wo', name=f'wo{ec}')
                        for h in range(HG):
                            nc.sync.dma_start(wo[:, h, :], woT[:, h, es])
                        for tt in range(NT):
                            pso = oppool.tile([P, 512], F32, tag='pso',
                                              name=f'pso_{tt}_{ec}')
                            for h in range(HG):
                                nc.tensor.matmul(pso[:], zT[h][:, tt * P:(tt + 1) * P],
                                                 wo[:, h, :],
                                                 start=(h == 0), stop=(h == HG - 1))
                            osb = ospool.tile([P, 512], F32, tag='osb',
                                              name=f'osb_{tt}_{ec}')
                            if tt % 2 == 0:
                                nc.scalar.copy(osb[:], pso[:])
                            else:
                                nc.vector.tensor_copy(osb[:], pso[:])
                            nc.sync.dma_start(out[tt][:, es], osb[:])

    nc.compile()
    return nc


def _host_inputs(x, Wqkv, Wo):
    """Build the 8 per-core input maps (bf16 weights/activations)."""
    bf16 = ml_dtypes.bfloat16
    # RoPE tables (match reference: float32 math)
    inv_freq = (1.0 / (BASE ** (np.arange(0, DH, 2, dtype=np.float32) / DH))).astype(np.float32)
    t = np.arange(S, dtype=np.float32)
    freqs = np.einsum('i,j->ij', t, inv_freq).astype(np.float32)   # [S, 64]
    emb = np.concatenate([freqs, freqs], axis=-1)                   # [S, 128]
    cos = np.cos(emb).astype(np.float32)
    sin = np.sin(emb).astype(np.float32)
    cosT = np.ascontiguousarray(cos.T)                              # [128, S]
    sinT = np.ascontiguousarray(sin.T)
    sinP = sinT.copy()
    sinP[0:64] = -sinP[0:64]

    # triangular causal mask [128, 128] bf16: keep iff k_rel <= q_rel
    maskT = (np.arange(P)[:, None] <= np.arange(P)[None, :]).astype(bf16)
    onescol = np.ones((P, 1), dtype=np.float32)

    in_maps = []
    for c in range(8):
        b, g = c // 2, c % 2
        heads = range(HG * g, HG * g + HG)
        x_b = x[b]                                       # [S, D]
        # [128 dpart, 4 tcn, 16 o, 512 tok]
        xT = np.ascontiguousarray(
            x_b.T.reshape(NDM, P, NC512, 512).transpose(1, 2, 0, 3)).astype(bf16)
        # Q then K feature blocks, one per head in group
        blocks = [Wqkv[h * DH:(h + 1) * DH] for h in heads] + \
                 [Wqkv[D + h * DH:D + (h + 1) * DH] for h in heads]
        wqkT = np.stack([
            np.ascontiguousarray(
                blk.T.reshape(NDM, P, P).transpose(1, 0, 2))    # [128, 16, 128]
            for blk in blocks
        ]).astype(bf16)                                          # [16, 128, 16, 128]
        Wv = np.concatenate([Wqkv[2 * D + h * DH:2 * D + (h + 1) * DH] for h in heads])
        wvT = np.ascontiguousarray(
            Wv.T.reshape(NDM, P, HG * P).transpose(1, 0, 2)).astype(bf16)  # [128, 16, 1024]
        Wog = Wo[:, g * HG * DH:(g + 1) * HG * DH]               # [D, 1024]
        woT = np.ascontiguousarray(
            Wog.T.reshape(HG, P, D).transpose(1, 0, 2)).astype(bf16)       # [128, 8, D]
        in_maps.append({
            'xT': xT, 'wqkT': wqkT, 'wvT': wvT, 'woT': woT,
            'cosT': cosT, 'sinP': sinP, 'maskT': maskT, 'onescol': onescol,
        })
    return in_maps


def kernel(x, Wqkv, Wo):
    from concourse.bass_utils import run_bass_kernel_spmd

    if 'nc' not in _CACHE:
        _CACHE['nc'] = _build_program()
    nc = _CACHE['nc']

    in_maps = _host_inputs(np.asarray(x, dtype=np.float32),
                           np.asarray(Wqkv, dtype=np.float32),
                           np.asarray(Wo, dtype=np.float32))
    res = run_bass_kernel_spmd(nc, in_maps, core_ids=list(range(8)))
    outs = [res.results[c]['out'].reshape(S, D) for c in range(8)]
    full = np.empty((B, S, D), dtype=np.float32)
    for b in range(B):
        full[b] = outs[2 * b] + outs[2 * b + 1]
    return full


# revision 13
# speedup vs baseline: 1.1490x; 1.1490x over previous
"""Distributed causal multi-head attention (RoPE) for 8 TRN2 NeuronCores.

Problem: B=4, S=2048, D=2048, H=16 heads, DH=128.
Sharding: 2D - data-parallel over the 4 batches x tensor-parallel over 2
head-groups of 8 heads (Megatron-style: Wqkv column-sharded per head
group, Wo row-sharded).  Core c handles batch c//2, head group c%2.
Each core returns a partial output projection [S, D]; the host sums the
two group partials per batch (the "all-reduce") and stacks batches.

v3: bf16 moving operands (fp32 PSUM accumulation), f32r stationaries
where SBUF allows (bf16 stationaries trigger a 4-wide fast-weight-load
whose XBUS traffic throttles concurrent matmuls from 2.4 to 2.0 GHz),
fully SBUF-resident (no DRAM spill), V-stationary PV:
  stage 1: QKV projection from xT (d-major, bf16).  Weights f32r
           stationary.  RoPE fused into the PSUM eviction for Q/K (DVE
           cross-partition reads, sign folded into the fp16 sin table);
           Q/K/V stay resident in SBUF as bf16.  x is DMA'd in
           token-chunk order so the first matmul chain starts early.
  stage 2: per head: K_h and V_h are restaged to f32r by DVE (cheap,
           hidden) so PE stationaries avoid FWL.  scoresT[k,q] =
           K-tile.T x Q (causally trimmed at 128 granularity), exp via
           ACT (scale fused) into bf16 tiles, triangular mask on
           diagonal blocks only.  PV is V-stationary: zT[dh,q] +=
           V_kt.T @ st (one LDWEIGHTS per K-tile, N=512 moving).
           Softmax denominator: DVE pairwise tree-sum of the exp'd
           tiles, a 1-column ones matmul for the partition reduction
           (deferred one chunk so the PE never waits on the DVE tree),
           DVE reciprocal, then partition_broadcast + normalize multiply
           both on the otherwise-idle GPSIMD engine.
  stage 3: output projection out = sum_h zT_h.T x WoT_h (N=512); zT is
           restaged to f32r by DVE at stage-3 entry.

Baseline (v1, fp32r + DRAM spill + st-stationary PV): ~727-736 us.
"""

import sys

if '/opt/trn_rl_repo' not in sys.path:
    sys.path.insert(0, '/opt/trn_rl_repo')

import math

import ml_dtypes
import numpy as np

B, S, D, H, DH = 4, 2048, 2048, 16, 128
BASE = 10000.0
P = 128
NT = S // P          # 16 token tiles of 128
NC512 = S // 512     # 4 token chunks of 512
NDM = D // P         # 16 d_model chunks
HG = 8               # heads per group
SCALE = 1.0 / math.sqrt(DH)

_CACHE = {}


def _build_program():
    import concourse.bacc as bacc
    import concourse.mybir as mybir
    from concourse.tile import TileContext

    F16 = mybir.dt.float16
    F32 = mybir.dt.float32
    F32R = mybir.dt.float32r
    BF16 = mybir.dt.bfloat16
    EXP = mybir.ActivationFunctionType.Exp

    nc = bacc.Bacc('TRN2', target_bir_lowering=False, debug=False, num_devices=8)

    # ---- DRAM I/O ----
    xT = nc.dram_tensor('xT', [P, NC512, NDM, 512], BF16, kind='ExternalInput').ap()
    wqkT = nc.dram_tensor('wqkT', [2 * HG, P, NDM, P], BF16, kind='ExternalInput').ap()
    wvT = nc.dram_tensor('wvT', [P, NDM, HG * P], BF16, kind='ExternalInput').ap()
    woT = nc.dram_tensor('woT', [P, HG, D], F32R, kind='ExternalInput').ap()
    cosT = nc.dram_tensor('cosT', [P, S], F16, kind='ExternalInput').ap()
    sinP = nc.dram_tensor('sinP', [P, S], F16, kind='ExternalInput').ap()
    maskT = nc.dram_tensor('maskT', [P, P], BF16, kind='ExternalInput').ap()
    onescol = nc.dram_tensor('onescol', [P, 1], F32R, kind='ExternalInput').ap()
    out = nc.dram_tensor('out', [NT, P, D], F32, kind='ExternalOutput').ap()

    with TileContext(nc) as tc:
        # Persistent across stages 1-2: Q/K feature blocks and V token tiles.
        with tc.tile_pool(name='qk', bufs=1) as qkpool, \
             tc.tile_pool(name='v', bufs=1) as vpool:
            qksb = [qkpool.tile([P, S], BF16, name=f'qk{fb}') for fb in range(2 * HG)]
            vsb = [vpool.tile([P, HG * P], BF16, name=f'v{tt}') for tt in range(NT)]

            # ================= stage 1: QKV projection =================
            with tc.tile_pool(name='s1x', bufs=1) as xpool:
                xsb = xpool.tile([P, NC512, NDM, 512], BF16)
                with tc.tile_pool(name='s1t', bufs=1) as tpool, \
                     tc.tile_pool(name='s1w', bufs=2) as wpool, \
                     tc.tile_pool(name='s1e', bufs=2) as epool, \
                     tc.tile_pool(name='s1p', bufs=4, space='PSUM') as ppool:
                    cos_sb = tpool.tile([P, S], F16)
                    sin_sb = tpool.tile([P, S], F16)
                    for fb in range(2 * HG):
                        wsb = wpool.tile([P, NDM, P], BF16, tag='w', name=f'wqk{fb}')
                        nc.sync.dma_start(wsb[:], wqkT[fb])
                        if fb == 0:
                            # x in token-chunk order so (fb0, tcn0) starts asap
                            nc.sync.dma_start(xsb[:, 0], xT[:, 0])
                            nc.sync.dma_start(cos_sb[:], cosT[:])
                            nc.sync.dma_start(sin_sb[:], sinP[:])
                            for tcn in range(1, NC512):
                                nc.sync.dma_start(xsb[:, tcn], xT[:, tcn])
                        for tcn in range(NC512):
                            ts = slice(tcn * 512, tcn * 512 + 512)
                            ps = ppool.tile([P, 512], F32, tag='pqk', name=f'pqk_{fb}_{tcn}')
                            for o in range(NDM):
                                nc.tensor.matmul(ps[:], wsb[:, o, :], xsb[:, tcn, o, :],
                                                 start=(o == 0), stop=(o == NDM - 1))
                            # RoPE fused eviction (sign folded in sinP)
                            t1 = epool.tile([P, 512], F32, tag='tt', name=f't1_{fb}_{tcn}')
                            t2 = epool.tile([P, 512], F32, tag='tt', name=f't2_{fb}_{tcn}')
                            nc.vector.tensor_mul(t1[:], ps[:], cos_sb[:, ts])
                            nc.vector.tensor_mul(t2[0:64, :], ps[64:128, :], sin_sb[0:64, ts])
                            nc.vector.tensor_mul(t2[64:128, :], ps[0:64, :], sin_sb[64:128, ts])
                            nc.vector.tensor_add(qksb[fb][:, ts], t1[:], t2[:])

                    # --- V blocks, token-major, N=256 ---
                    for vc in range(4):
                        vs = slice(vc * 256, vc * 256 + 256)
                        wv = wpool.tile([P, NDM, 256], BF16, tag='wv', name=f'wv{vc}')
                        for o in range(NDM):
                            nc.sync.dma_start(wv[:, o, :], wvT[:, o, vs])
                        for tt in range(NT):
                            psv = ppool.tile([P, 256], F32, tag='pv', name=f'pv_{vc}_{tt}')
                            for o in range(NDM):
                                nc.tensor.matmul(
                                    psv[:],
                                    xsb[:, tt // 4, o, (tt % 4) * P:(tt % 4 + 1) * P],
                                    wv[:, o, :],
                                    start=(o == 0), stop=(o == NDM - 1))
                            nc.scalar.copy(vsb[tt][:, vs], psv[:])

            # ================= stage 2 + 3 =================
            with tc.tile_pool(name='zt', bufs=1) as ztpool:
                zT = [ztpool.tile([P, S], BF16, name=f'zT{h}') for h in range(HG)]

                with tc.tile_pool(name='s2c', bufs=1) as cpool, \
                     tc.tile_pool(name='s2st', bufs=2) as stpool, \
                     tc.tile_pool(name='s2dn', bufs=5) as dnpool, \
                     tc.tile_pool(name='s2dp', bufs=3) as dppool, \
                     tc.tile_pool(name='s2rc', bufs=2) as rcpool, \
                     tc.tile_pool(name='s2ps', bufs=4, space='PSUM') as sppool, \
                     tc.tile_pool(name='s2pz', bufs=3, space='PSUM') as zppool, \
                     tc.tile_pool(name='s2pd', bufs=1, space='PSUM') as dpppool:
                    msk = cpool.tile([P, P], BF16)
                    nc.sync.dma_start(msk[:], maskT[:])
                    ones_sb = cpool.tile([P, 1], F32R)
                    nc.sync.dma_start(ones_sb[:], onescol[:])

                    pend = []

                    def flush_den_a(item):
                        ph, pqr, zps, dp = item
                        den_ps = dpppool.tile([1, 512], F32, tag='denp',
                                              name=f'den_{ph}_{pqr}')
                        nc.tensor.matmul(den_ps[:], ones_sb[:], dp[:],
                                         start=True, stop=True)
                        rcp = rcpool.tile([1, 512], F32, tag='rcp',
                                          name=f'rcp_{ph}_{pqr}')
                        nc.vector.reciprocal(rcp[:], den_ps[:])
                        rcpb = rcpool.tile([P, 512], F32, tag='rcpb',
                                           name=f'rcpb_{ph}_{pqr}')
                        nc.gpsimd.partition_broadcast(rcpb[:], rcp[:])
                        item.append(rcpb)

                    def flush_den_b():
                        ph, pqr, zps, dp, rcpb = pend.pop(0)
                        nc.vector.tensor_mul(zT[ph][:, pqr * 512:(pqr + 1) * 512],
                                             zps[:], rcpb[:])

                    for h in range(HG):
                        qt_h = qksb[h]
                        kt_h = qksb[HG + h]
                        st = [[None] * NT for _ in range(NC512)]

                        def emit_qk(qr, h=h, qt_h=qt_h, kt_h=kt_h, st=st):
                            base = qr * 512
                            for kt in range(4 * qr + 4):
                                d = kt - 4 * qr
                                qoff = 0 if d < 0 else 128 * d
                                sps = sppool.tile([P, 512], F32, tag='sps',
                                                  name=f'sps_{h}_{qr}_{kt}')
                                nc.tensor.matmul(sps[:, qoff:512],
                                                 kt_h[:, kt * P:(kt + 1) * P],
                                                 qt_h[:, base + qoff:base + 512],
                                                 start=True, stop=True)
                                stt = stpool.tile([P, 512], BF16, tag=f'st{kt}',
                                                  name=f'st_{h}_{qr}_{kt}')
                                nc.scalar.activation(stt[:, qoff:512], sps[:, qoff:512],
                                                     EXP, scale=SCALE)
                                if d >= 0:
                                    # triangular mask on the diagonal 128-block only
                                    nc.vector.tensor_mul(stt[:, qoff:qoff + P],
                                                         stt[:, qoff:qoff + P], msk[:])
                                st[qr][kt] = stt

                        emit_qk(0)
                        for qr in range(NC512):
                            if qr + 1 < NC512:
                                emit_qk(qr + 1)   # QK(qr+1) on PE while ACT exps it
                            nkt = 4 * qr + 4
                            # --- PV, V-stationary: zT accum [dh, q] ---
                            zps = zppool.tile([P, 512], F32, tag='zps',
                                              name=f'zps_{h}_{qr}')
                            for kt in range(nkt):
                                d = kt - 4 * qr
                                qoff = 0 if d < 0 else 128 * d
                                nc.tensor.matmul(zps[:, qoff:512],
                                                 vsb[kt][:, h * P:(h + 1) * P],
                                                 st[qr][kt][:, qoff:512],
                                                 start=(kt == 0), stop=(kt == nkt - 1))
                            # --- denominator partials: quad-block bf16 sums folded
                            #     into an f32 chain, then the 3 trimmed tiles ---
                            fulls = [st[qr][kt][:, 0:512] for kt in range(4 * qr + 1)]
                            items = []
                            lvl = 0
                            i = 0
                            while i + 3 < len(fulls):
                                v1 = dnpool.tile([P, 512], BF16, tag='dnb',
                                                 name=f'dn_{h}_{qr}_{lvl}a')
                                nc.vector.tensor_add(v1[:], fulls[i], fulls[i + 1])
                                v2 = dnpool.tile([P, 512], BF16, tag='dnb',
                                                 name=f'dn_{h}_{qr}_{lvl}b')
                                nc.vector.tensor_add(v2[:], fulls[i + 2], fulls[i + 3])
                                w = dnpool.tile([P, 512], BF16, tag='dnb',
                                                name=f'dn_{h}_{qr}_{lvl}w')
                                nc.vector.tensor_add(w[:], v1[:], v2[:])
                                items.append(w[:])
                                lvl += 1
                                i += 4
                            items.extend(fulls[i:])
                            dp = dppool.tile([P, 512], F32R, tag='dp',
                                             name=f'dp_{h}_{qr}_0')
                            if len(items) == 1:
                                nc.vector.tensor_copy(dp[:], items[0])
                            else:
                                nc.vector.tensor_add(dp[:], items[0], items[1])
                                for j, it in enumerate(items[2:]):
                                    dpn = dppool.tile([P, 512], F32R, tag='dp',
                                                      name=f'dp_{h}_{qr}_{j + 1}')
                                    nc.vector.tensor_add(dpn[:], dp[:], it)
                                    dp = dpn
                            for dd in range(1, 4):
                                qoff = 128 * dd
                                nc.vector.tensor_add(dp[:, qoff:512], dp[:, qoff:512],
                                                     st[qr][4 * qr + dd][:, qoff:512])
                            pend.append([h, qr, zps, dp])
                            # deferred: the PE-side ones-matmul never waits on the
                            # DVE tree; the DVE multiply never waits on the GPSIMD
                            # broadcast round-trip
                            if len(pend) > 1:
                                flush_den_a(pend[-2])
                            if len(pend) > 2:
                                flush_den_b()
                    flush_den_a(pend[-1])
                    while pend:
                        flush_den_b()

                # ================= stage 3: output projection =================
                # token-tile outer; zT restaged to f32r per tile (cheap DVE copy)
                with tc.tile_pool(name='s3z', bufs=2) as zfpool, \
                     tc.tile_pool(name='s3o', bufs=3) as ospool, \
                     tc.tile_pool(name='s3w', bufs=1) as wopool, \
                     tc.tile_pool(name='s3p', bufs=4, space='PSUM') as oppool:
                    wo = wopool.tile([P, HG, D], F32R)
                    for h in range(HG):
                        nc.sync.dma_start(wo[:, h, :], woT[:, h, :])
                    for tt in range(NT):
                        zf = zfpool.tile([P, HG, P], F32R, tag='zf', name=f'zf{tt}')
                        for h in range(HG):
                            nc.vector.tensor_copy(zf[:, h, :],
                                                  zT[h][:, tt * P:(tt + 1) * P])
                        for ec in range(NC512):
                            es = slice(ec * 512, ec * 512 + 512)
                            pso = oppool.tile([P, 512], F32, tag='pso',
                                              name=f'pso_{tt}_{ec}')
                            for h in range(HG):
                                nc.tensor.matmul(pso[:], zf[:, h, :],
                                                 wo[:, h, es],
                                                 start=(h == 0), stop=(h == HG - 1))
                            osb = ospool.tile([P, 512], F32, tag='osb',
                                              name=f'osb_{tt}_{ec}')
                            if ec % 2 == 0:
                                nc.scalar.copy(osb[:], pso[:])
                            else:
                                nc.vector.tensor_copy(osb[:], pso[:])
                            nc.sync.dma_start(out[tt][:, es], osb[:])

    nc.compile()
    return nc


def _host_inputs(x, Wqkv, Wo):
    """Build the 8 per-core input maps."""
    bf16 = ml_dtypes.bfloat16
    # RoPE tables (match reference: float32 math; fp16 is near-exact on [-1,1])
    inv_freq = (1.0 / (BASE ** (np.arange(0, DH, 2, dtype=np.float32) / DH))).astype(np.float32)
    t = np.arange(S, dtype=np.float32)
    freqs = np.einsum('i,j->ij', t, inv_freq).astype(np.float32)   # [S, 64]
    emb = np.concatenate([freqs, freqs], axis=-1)                   # [S, 128]
    cos = np.cos(emb)
    sin = np.sin(emb)
    cosT = np.ascontiguousarray(cos.T).astype(np.float16)           # [128, S]
    sinT = np.ascontiguousarray(sin.T)
    sinP = sinT.copy()
    sinP[0:64] = -sinP[0:64]
    sinP = sinP.astype(np.float16)

    # triangular causal mask [128, 128] bf16: keep iff k_rel <= q_rel
    maskT = (np.arange(P)[:, None] <= np.arange(P)[None, :]).astype(bf16)
    onescol = np.ones((P, 1), dtype=np.float32)

    in_maps = []
    for c in range(8):
        b, g = c // 2, c % 2
        heads = range(HG * g, HG * g + HG)
        x_b = x[b]                                       # [S, D]
        # [128 dpart, 4 tcn, 16 o, 512 tok]
        xT = np.ascontiguousarray(
            x_b.T.reshape(NDM, P, NC512, 512).transpose(1, 2, 0, 3)).astype(bf16)
        # Q then K feature blocks, one per head in group
        blocks = [Wqkv[h * DH:(h + 1) * DH] for h in heads] + \
                 [Wqkv[D + h * DH:D + (h + 1) * DH] for h in heads]
        wqkT = np.stack([
            np.ascontiguousarray(
                blk.T.reshape(NDM, P, P).transpose(1, 0, 2))    # [128, 16, 128]
            for blk in blocks
        ]).astype(bf16)                                          # [16, 128, 16, 128]
        Wv = np.concatenate([Wqkv[2 * D + h * DH:2 * D + (h + 1) * DH] for h in heads])
        wvT = np.ascontiguousarray(
            Wv.T.reshape(NDM, P, HG * P).transpose(1, 0, 2)).astype(bf16)  # [128, 16, 1024]
        Wog = Wo[:, g * HG * DH:(g + 1) * HG * DH]               # [D, 1024]
        woT = np.ascontiguousarray(
            Wog.T.reshape(HG, P, D).transpose(1, 0, 2))          # [128, 8, D] f32
        in_maps.append({
            'xT': xT, 'wqkT': wqkT, 'wvT': wvT, 'woT': woT,
            'cosT': cosT, 'sinP': sinP, 'maskT': maskT, 'onescol': onescol,
        })
    return in_maps


def kernel(x, Wqkv, Wo):
    from concourse.bass_utils import run_bass_kernel_spmd

    if 'nc' not in _CACHE:
        _CACHE['nc'] = _build_program()
    nc = _CACHE['nc']

    in_maps = _host_inputs(np.asarray(x, dtype=np.float32),
                           np.asarray(Wqkv, dtype=np.float32),
                           np.asarray(Wo, dtype=np.float32))
    res = run_bass_kernel_spmd(nc, in_maps, core_ids=list(range(8)))
    outs = [res.results[c]['out'].reshape(S, D) for c in range(8)]
    full = np.empty((B, S, D), dtype=np.float32)
    for b in range(B):
        full[b] = outs[2 * b] + outs[2 * b + 1]
    return full
